# revision 4
# baseline (speedup 1.0000x reference)
"""LIF spike layer on 8 Trainium2 NeuronCores — PE-packed spike output.

Reference recurrence over T=16 (elementwise per neuron):
    u_t     = 0.5*mem_t + 0.5*x_t
    s_t     = (u_t > 1.0)
    mem_t+1 = u_t * (1 - s_t)

Sharding: batch axis (axis 1, B=32) split 4-per-core across 8 cores; zero
communication. Per core each timestep is a [128 partitions x 4096 free] slab
processed as two 2048-column chains so every engine always has runnable work.

Doubled-state formulation (M := 2*mem, V := 2*u = 0.5*M + x), per step:
    DVE (VectorE): V  = (M * 0.5) + x     scalar_tensor_tensor, fp32 (exact)
    DVE (VectorE): M' = (V <= 2) * V      scalar_tensor_tensor, fp32 (exact)
    ACT (ScalarE): g  = Sign(1 - 0.5*V)   in {-1,0,+1} -> bf16 (exact;
                                          spike <=> g == -1, ties at V==2
                                          give g==0 = no spike, as required)
    PE  (TensorE): acc[:,bank] += diag(4^j).T @ g    (bf16 matmul, fp32 PSUM)

Spike-output packing (the point of this variant): instead of DMAing one
fp8 spike map per timestep (8 MiB/core), TensorE accumulates base-4 digits
in PSUM: with h := g+1 in {0,1,2} (h==0 <=> spike), acc = sum_j 4^j*g_{t0+j}
and acc + sum_j 4^j is a base-4 integer whose j-th digit is h_j. Digits are
< 4 so no carries, all magnitudes < 2^15 so fp32 PSUM accumulation is exact
and the result round-trips through an int16 store. Phases: t=0..7 -> pk0
(int16, via ScalarE PSUM->SBUF copy), t=8..14 -> pk1 (int16), and t=15 is
emitted directly as fp8 Sign (keeps the kernel tail off the PE/PSUM path).
Host decodes digits. Per-core HBM traffic drops from ~40 MiB (fp8 spikes)
to ~35 MiB (32 in + 0.5 weights + 2.5 out) — and HBM *writes*, which are
several times more expensive per byte than reads on this part, drop 3.2x.

Exactness: all device arithmetic on the V/M path is the same fp32
scalar_tensor_tensor chain as the reference (single rounding per step,
power-of-two scalings exact), Sign is exact including the V==2 tie, bf16
holds {-1,0,1} and 4^j (j<8) exactly, PSUM products/sums are exact
integers < 2^15, and int16 conversion of exact integers is exact. Verified
bit-for-bit: 0/67,108,864 mismatches, including planted threshold ties.

Measured (8 cores concurrently, hardware repeat-loop slope R=4 vs R=604,
min of 8 dispatches per point, reproduced twice within 1%): 147,697 ns /
149,163 ns per iteration incl. ~2.5us loop-barrier overhead, vs 177,832 ns
for the fp8-output baseline under the identical rig (1.20x); same-R
loop totals confirm this config is the fastest variant tried. Engine
budget per core at measured rates: DVE 60 STT ops ~114us (bound),
ACT ~72us, PE ~35us, DMA ~35 MiB ~100us, all overlapped.
"""

import numpy as np

T = 16
B = 32
CDIM = 128
H = 32
W = 32
NCORES = 8
B_LOC = B // NCORES              # 4
PART = 128
FREE = B_LOC * CDIM * H * W // PART   # 4096
CHUNK = 2048
NCH = FREE // CHUNK
TSPLIT = 8                       # phase A t=0..7, phase B t=8..14, t=15 direct fp8

_NC = None


def _build_wd():
    """Stationary weights: wd[:, t*128:(t+1)*128] = diag(4^(t % TSPLIT))."""
    import ml_dtypes
    wd = np.zeros((PART, T * PART), np.float32)
    for t in range(T):
        j = t if t < TSPLIT else t - TSPLIT
        wd[np.arange(PART), t * PART + np.arange(PART)] = float(4 ** j)
    return wd.astype(ml_dtypes.bfloat16)


def build(num_devices=NCORES, internal_io=False, repeats=1):
    import concourse.bacc as bacc
    import concourse.tile as tile
    import concourse.mybir as mybir

    nc = bacc.Bacc("TRN2", debug=False, target_bir_lowering=False,
                   num_devices=num_devices)
    fp32 = mybir.dt.float32
    bf16 = mybir.dt.bfloat16
    fp8 = mybir.dt.float8e4
    i16 = mybir.dt.int16
    Alu = mybir.AluOpType
    Act = mybir.ActivationFunctionType

    kin = "Internal" if internal_io else "ExternalInput"
    kout = "Internal" if internal_io else "ExternalOutput"
    x_d = nc.dram_tensor("x", [T, PART, FREE], fp32, kind=kin).ap()
    wd_d = nc.dram_tensor("wd", [PART, T * PART], bf16, kind=kin).ap()
    p0_d = nc.dram_tensor("pk0", [PART, FREE], i16, kind=kout).ap()
    p1_d = nc.dram_tensor("pk1", [PART, FREE], i16, kind=kout).ap()
    s15_d = nc.dram_tensor("s15", [PART, FREE], fp8, kind=kout).ap()
    if internal_io:
        xs_d = nc.dram_tensor("xs", [PART, 16], fp32, kind="ExternalInput").ap()
        os_d = nc.dram_tensor("os", [PART, 16], fp32, kind="ExternalOutput").ap()

    with tile.TileContext(nc) as tc:
        with (
            tc.tile_pool(name="cp", bufs=1) as cp,
            tc.tile_pool(name="xp", bufs=5) as xp,
            tc.tile_pool(name="vp", bufs=4) as vp,
            tc.tile_pool(name="mp", bufs=1) as mp,
            tc.tile_pool(name="gp", bufs=4) as gp,
            tc.tile_pool(name="sp", bufs=2) as sp,
            tc.tile_pool(name="aq", bufs=1, space="PSUM") as aq,
        ):
            # weights go on the ScalarE HWDGE queue so they don't delay the
            # first x-tile loads on the sync queue
            wd = cp.tile([PART, T * PART], bf16, name="wd")
            nc.scalar.dma_start(wd[:], wd_d)
            if internal_io:
                small = cp.tile([PART, 16], fp32, name="small")
                nc.sync.dma_start(small[:], xs_d)

            ms = [mp.tile([PART, CHUNK], fp32, tag=f"m{c}", name=f"m{c}")
                  for c in range(NCH)]
            accs = [aq.tile([PART, CHUNK], fp32, tag=f"acc{c}", name=f"acc{c}")
                    for c in range(NCH)]

            def body():
                for t in range(T):
                    for c in range(NCH):
                        sl = slice(c * CHUNK, (c + 1) * CHUNK)
                        xt = xp.tile([PART, CHUNK], fp32)
                        nc.sync.dma_start(xt[:], x_d[t, :, sl])
                        if t == 0:
                            v = xt          # M_0 = 0 -> V_0 = x_0
                        else:
                            v = vp.tile([PART, CHUNK], fp32)
                            nc.vector.scalar_tensor_tensor(
                                v[:], ms[c][:], 0.5, xt[:], Alu.mult, Alu.add)
                        if t < T - 1:
                            nc.vector.scalar_tensor_tensor(
                                ms[c][:], v[:], 2.0, v[:], Alu.is_le, Alu.mult)
                        if t == T - 1:
                            # final step: emit spikes directly as fp8 (keeps
                            # the kernel tail off the PE/PSUM/copy path)
                            st = gp.tile([PART, CHUNK], fp8)
                            nc.scalar.activation(st[:], v[:], Act.Sign,
                                                 bias=1.0, scale=-0.5)
                            nc.sync.dma_start(s15_d[:, sl], st[:])
                            continue
                        g = gp.tile([PART, CHUNK], bf16)
                        nc.scalar.activation(g[:], v[:], Act.Sign,
                                             bias=1.0, scale=-0.5)
                        for b in range(CHUNK // 512):
                            bs = slice(b * 512, (b + 1) * 512)
                            nc.tensor.matmul(
                                accs[c][:, bs],
                                wd[:, t * PART:(t + 1) * PART],
                                g[:, bs],
                                start=(t in (0, TSPLIT)),
                                stop=(t in (TSPLIT - 1, T - 2)))
                        if t == TSPLIT - 1:
                            s0 = sp.tile([PART, CHUNK], i16)
                            nc.scalar.copy(s0[:], accs[c][:])
                            nc.sync.dma_start(p0_d[:, sl], s0[:])
                        if t == T - 2:
                            s1 = sp.tile([PART, CHUNK], i16)
                            nc.scalar.copy(s1[:], accs[c][:])
                            nc.sync.dma_start(p1_d[:, sl], s1[:])

            if repeats == 1:
                body()
            else:
                with tc.For_i(0, repeats):
                    body()
            if internal_io:
                nc.sync.dma_start(os_d, small[:])
    nc.compile()
    return nc


def _get_nc():
    global _NC
    if _NC is None:
        _NC = build()
    return _NC


CONST_A = sum(4 ** j for j in range(TSPLIT))        # 21845
CONST_B = sum(4 ** j for j in range(T - 1 - TSPLIT))  # 5461


def _decode(pk0, pk1, s15):
    """pk0 int16 (t=0..7), pk1 int16 (t=8..14), s15 fp8 (t=15)."""
    u0 = (np.asarray(pk0).astype(np.int64) + CONST_A)
    u1 = (np.asarray(pk1).astype(np.int64) + CONST_B)
    out = np.empty((T, PART, FREE), np.float32)
    for j in range(TSPLIT):
        out[j] = (((u0 >> (2 * j)) & 3) == 0).astype(np.float32)
    for j in range(T - 1 - TSPLIT):
        out[TSPLIT + j] = (((u1 >> (2 * j)) & 3) == 0).astype(np.float32)
    g15 = np.asarray(s15).astype(np.float32)
    out[T - 1] = np.maximum(-g15, np.float32(0.0))
    return out


def kernel(x):
    from concourse.bass_utils import run_bass_kernel_spmd

    x = np.asarray(x)
    assert x.shape == (T, B, CDIM, H, W) and x.dtype == np.float32
    nc = _get_nc()
    wd = _build_wd()
    in_maps = []
    for c in range(NCORES):
        xc = np.ascontiguousarray(x[:, c * B_LOC:(c + 1) * B_LOC])
        in_maps.append({"x": xc.reshape(T, PART, FREE), "wd": wd})
    res = run_bass_kernel_spmd(nc, in_maps, list(range(NCORES))).results
    parts = []
    for r in res:
        spikes = _decode(r["pk0"], r["pk1"], r["s15"])
        parts.append(spikes.reshape(T, B_LOC, CDIM, H, W))
    return np.concatenate(parts, axis=1)


def measure(r_lo=4, r_hi=604, reps=8, ncores=NCORES):
    """HW per-iteration time via repeat-loop slope (internal-DRAM variant).

    Runs the full per-core pipeline (with Internal-DRAM x/outputs, tiny
    external I/O) R times in a hardware For_i loop on all `ncores` cores at
    once so HBM bandwidth sharing matches the production run. The slope
    between the two repeat counts cancels the axon-tunnel dispatch constant.
    """
    import time
    from concourse.bass_utils import run_bass_kernel_spmd
    xs = np.zeros((PART, 16), np.float32)
    in_maps = [{"xs": xs} for _ in range(ncores)]
    times = {}
    for R in (r_lo, r_hi):
        nc = build(num_devices=ncores, internal_io=True, repeats=R)
        ts = []
        for _ in range(reps):
            t0 = time.time()
            run_bass_kernel_spmd(nc, in_maps, list(range(ncores)))
            ts.append(time.time() - t0)
        times[R] = min(ts)
        print(f"  full R={R}: min {times[R]*1e3:.1f} ms  all "
              f"{[f'{t*1e3:.0f}' for t in ts]}", flush=True)
    slope = (times[r_hi] - times[r_lo]) / (r_hi - r_lo) * 1e9
    print(f"== full kernel ({ncores} cores): {slope:.0f} ns/iter", flush=True)
    return slope


# revision 7
# speedup vs baseline: 1.1783x; 1.1783x over previous
"""LIF spike layer on 8 Trainium2 NeuronCores — PE-packed spike output.

Reference recurrence over T=16 (elementwise per neuron):
    u_t     = 0.5*mem_t + 0.5*x_t
    s_t     = (u_t > 1.0)
    mem_t+1 = u_t * (1 - s_t)

Sharding: batch axis (axis 1, B=32) split 4-per-core across 8 cores; zero
communication. Per core each timestep is a [128 partitions x 4096 free] slab
processed as two 2048-column chains so every engine always has runnable work.

Doubled-state formulation (M := 2*mem, V := 2*u = 0.5*M + x), per step:
    DVE (VectorE): V  = (M * 0.5) + x     scalar_tensor_tensor, fp32 (exact)
    DVE (VectorE): M' = (V <= 2) * V      scalar_tensor_tensor, fp32 (exact)
    ACT (ScalarE): g  = Sign(1 - 0.5*V)   in {-1,0,+1} -> bf16 (exact;
                                          spike <=> g == -1, ties at V==2
                                          give g==0 = no spike, as required)
    PE  (TensorE): acc[:,bank] += diag(4^j).T @ g    (bf16 matmul, fp32 PSUM)

Spike-output packing (the point of this variant): instead of DMAing one
fp8 spike map per timestep (8 MiB/core), TensorE accumulates base-4 digits
in PSUM: with h := g+1 in {0,1,2} (h==0 <=> spike), acc = sum_j 4^j*g_{t0+j}
and acc + sum_j 4^j is a base-4 integer whose j-th digit is h_j. Digits are
< 4 so no carries, all magnitudes < 2^15 so fp32 PSUM accumulation is exact
and the result round-trips through an int16 store. Phases: t=0..7 -> pk0
(int16, via ScalarE PSUM->SBUF copy), t=8..14 -> pk1 (int16), and t=15 is
emitted directly as fp8 Sign (keeps the kernel tail off the PE/PSUM path).
Host decodes digits. Per-core HBM traffic drops from ~40 MiB (fp8 spikes)
to ~35 MiB (32 in + 0.5 weights + 2.5 out) — and HBM *writes*, which are
several times more expensive per byte than reads on this part, drop 3.2x.

Exactness: all device arithmetic on the V/M path is the same fp32
scalar_tensor_tensor chain as the reference (single rounding per step,
power-of-two scalings exact), Sign is exact including the V==2 tie, bf16
holds {-1,0,1} and 4^j (j<8) exactly, PSUM products/sums are exact
integers < 2^15, and int16 conversion of exact integers is exact. Verified
bit-for-bit: 0/67,108,864 mismatches, including planted threshold ties.

Output DMAs are issued on the ScalarE HWDGE ring (sync ring carries only
input reads): HBM writes otherwise queue ahead of later x-tile reads and
starve the DVE late in the kernel — worth ~22us/iteration measured.

Measured (8 cores concurrently, hardware repeat-loop slope R=4 vs R=604,
min of 6-8 dispatches per point, incl. ~2.5us loop-barrier overhead):
125,344 ns/iteration (this config; 147,697/149,163 without the output
queue split), vs 177,832 ns for the fp8-output baseline under the
identical rig (1.42x). Engine budget per core at measured rates: DVE 60
STT ops ~114us (bound), ACT ~72us, PE ~35us, DMA ~35 MiB, all overlapped.
"""

import numpy as np

T = 16
B = 32
CDIM = 128
H = 32
W = 32
NCORES = 8
B_LOC = B // NCORES              # 4
PART = 128
FREE = B_LOC * CDIM * H * W // PART   # 4096
CHUNK = 2048
NCH = FREE // CHUNK
TSPLIT = 8                       # phase A t=0..7, phase B t=8..14, t=15 direct fp8

_NC = None


def _build_wd():
    """Stationary weights: wd[:, t*128:(t+1)*128] = diag(4^(t % TSPLIT))."""
    import ml_dtypes
    wd = np.zeros((PART, T * PART), np.float32)
    for t in range(T):
        j = t if t < TSPLIT else t - TSPLIT
        wd[np.arange(PART), t * PART + np.arange(PART)] = float(4 ** j)
    return wd.astype(ml_dtypes.bfloat16)


def build(num_devices=NCORES, internal_io=False, repeats=1):
    import concourse.bacc as bacc
    import concourse.tile as tile
    import concourse.mybir as mybir

    nc = bacc.Bacc("TRN2", debug=False, target_bir_lowering=False,
                   num_devices=num_devices)
    fp32 = mybir.dt.float32
    bf16 = mybir.dt.bfloat16
    fp8 = mybir.dt.float8e4
    i16 = mybir.dt.int16
    Alu = mybir.AluOpType
    Act = mybir.ActivationFunctionType

    kin = "Internal" if internal_io else "ExternalInput"
    kout = "Internal" if internal_io else "ExternalOutput"
    x_d = nc.dram_tensor("x", [T, PART, FREE], fp32, kind=kin).ap()
    wd_d = nc.dram_tensor("wd", [PART, T * PART], bf16, kind=kin).ap()
    p0_d = nc.dram_tensor("pk0", [PART, FREE], i16, kind=kout).ap()
    p1_d = nc.dram_tensor("pk1", [PART, FREE], i16, kind=kout).ap()
    s15_d = nc.dram_tensor("s15", [PART, FREE], fp8, kind=kout).ap()
    if internal_io:
        xs_d = nc.dram_tensor("xs", [PART, 16], fp32, kind="ExternalInput").ap()
        os_d = nc.dram_tensor("os", [PART, 16], fp32, kind="ExternalOutput").ap()

    with tile.TileContext(nc) as tc:
        with (
            tc.tile_pool(name="cp", bufs=1) as cp,
            tc.tile_pool(name="xp", bufs=5) as xp,
            tc.tile_pool(name="vp", bufs=4) as vp,
            tc.tile_pool(name="mp", bufs=1) as mp,
            tc.tile_pool(name="gp", bufs=4) as gp,
            tc.tile_pool(name="sp", bufs=2) as sp,
            tc.tile_pool(name="aq", bufs=1, space="PSUM") as aq,
        ):
            # weights go on the ScalarE HWDGE queue so they don't delay the
            # first x-tile loads on the sync queue
            wd = cp.tile([PART, T * PART], bf16, name="wd")
            nc.scalar.dma_start(wd[:], wd_d)
            if internal_io:
                small = cp.tile([PART, 16], fp32, name="small")
                nc.sync.dma_start(small[:], xs_d)

            ms = [mp.tile([PART, CHUNK], fp32, tag=f"m{c}", name=f"m{c}")
                  for c in range(NCH)]
            accs = [aq.tile([PART, CHUNK], fp32, tag=f"acc{c}", name=f"acc{c}")
                    for c in range(NCH)]

            def body():
                for t in range(T):
                    for c in range(NCH):
                        sl = slice(c * CHUNK, (c + 1) * CHUNK)
                        xt = xp.tile([PART, CHUNK], fp32)
                        nc.sync.dma_start(xt[:], x_d[t, :, sl])
                        if t == 0:
                            v = xt          # M_0 = 0 -> V_0 = x_0
                        else:
                            v = vp.tile([PART, CHUNK], fp32)
                            nc.vector.scalar_tensor_tensor(
                                v[:], ms[c][:], 0.5, xt[:], Alu.mult, Alu.add)
                        if t < T - 1:
                            nc.vector.scalar_tensor_tensor(
                                ms[c][:], v[:], 2.0, v[:], Alu.is_le, Alu.mult)
                        if t == T - 1:
                            # final step: emit spikes directly as fp8 (keeps
                            # the kernel tail off the PE/PSUM/copy path)
                            st = gp.tile([PART, CHUNK], fp8)
                            nc.scalar.activation(st[:], v[:], Act.Sign,
                                                 bias=1.0, scale=-0.5)
                            nc.scalar.dma_start(s15_d[:, sl], st[:])
                            continue
                        g = gp.tile([PART, CHUNK], bf16)
                        nc.scalar.activation(g[:], v[:], Act.Sign,
                                             bias=1.0, scale=-0.5)
                        for b in range(CHUNK // 512):
                            bs = slice(b * 512, (b + 1) * 512)
                            nc.tensor.matmul(
                                accs[c][:, bs],
                                wd[:, t * PART:(t + 1) * PART],
                                g[:, bs],
                                start=(t in (0, TSPLIT)),
                                stop=(t in (TSPLIT - 1, T - 2)))
                        if t == TSPLIT - 1:
                            s0 = sp.tile([PART, CHUNK], i16)
                            nc.scalar.copy(s0[:], accs[c][:])
                            nc.scalar.dma_start(p0_d[:, sl], s0[:])
                        if t == T - 2:
                            s1 = sp.tile([PART, CHUNK], i16)
                            nc.scalar.copy(s1[:], accs[c][:])
                            nc.scalar.dma_start(p1_d[:, sl], s1[:])

            if repeats == 1:
                body()
            else:
                with tc.For_i(0, repeats):
                    body()
            if internal_io:
                nc.sync.dma_start(os_d, small[:])
    nc.compile()
    return nc


def _get_nc():
    global _NC
    if _NC is None:
        _NC = build()
    return _NC


CONST_A = sum(4 ** j for j in range(TSPLIT))        # 21845
CONST_B = sum(4 ** j for j in range(T - 1 - TSPLIT))  # 5461


def _decode(pk0, pk1, s15):
    """pk0 int16 (t=0..7), pk1 int16 (t=8..14), s15 fp8 (t=15)."""
    u0 = (np.asarray(pk0).astype(np.int64) + CONST_A)
    u1 = (np.asarray(pk1).astype(np.int64) + CONST_B)
    out = np.empty((T, PART, FREE), np.float32)
    for j in range(TSPLIT):
        out[j] = (((u0 >> (2 * j)) & 3) == 0).astype(np.float32)
    for j in range(T - 1 - TSPLIT):
        out[TSPLIT + j] = (((u1 >> (2 * j)) & 3) == 0).astype(np.float32)
    g15 = np.asarray(s15).astype(np.float32)
    out[T - 1] = np.maximum(-g15, np.float32(0.0))
    return out


def kernel(x):
    from concourse.bass_utils import run_bass_kernel_spmd

    x = np.asarray(x)
    assert x.shape == (T, B, CDIM, H, W) and x.dtype == np.float32
    nc = _get_nc()
    wd = _build_wd()
    in_maps = []
    for c in range(NCORES):
        xc = np.ascontiguousarray(x[:, c * B_LOC:(c + 1) * B_LOC])
        in_maps.append({"x": xc.reshape(T, PART, FREE), "wd": wd})
    res = run_bass_kernel_spmd(nc, in_maps, list(range(NCORES))).results
    parts = []
    for r in res:
        spikes = _decode(r["pk0"], r["pk1"], r["s15"])
        parts.append(spikes.reshape(T, B_LOC, CDIM, H, W))
    return np.concatenate(parts, axis=1)


def measure(r_lo=4, r_hi=604, reps=8, ncores=NCORES):
    """HW per-iteration time via repeat-loop slope (internal-DRAM variant).

    Runs the full per-core pipeline (with Internal-DRAM x/outputs, tiny
    external I/O) R times in a hardware For_i loop on all `ncores` cores at
    once so HBM bandwidth sharing matches the production run. The slope
    between the two repeat counts cancels the axon-tunnel dispatch constant.
    """
    import time
    from concourse.bass_utils import run_bass_kernel_spmd
    xs = np.zeros((PART, 16), np.float32)
    in_maps = [{"xs": xs} for _ in range(ncores)]
    times = {}
    for R in (r_lo, r_hi):
        nc = build(num_devices=ncores, internal_io=True, repeats=R)
        ts = []
        for _ in range(reps):
            t0 = time.time()
            run_bass_kernel_spmd(nc, in_maps, list(range(ncores)))
            ts.append(time.time() - t0)
        times[R] = min(ts)
        print(f"  full R={R}: min {times[R]*1e3:.1f} ms  all "
              f"{[f'{t*1e3:.0f}' for t in ts]}", flush=True)
    slope = (times[r_hi] - times[r_lo]) / (r_hi - r_lo) * 1e9
    print(f"== full kernel ({ncores} cores): {slope:.0f} ns/iter", flush=True)
    return slope
